# revision 6
# baseline (speedup 1.0000x reference)
"""Column-wise RMS normalization on 8 Trainium2 NeuronCores.

Computes y = x * rsqrt(sum(x*x, axis=0) + eps) for x [32768, 2048] f32.

Sharding: column-parallel — each core owns a contiguous block of 256
columns, making the per-column sum-of-squares entirely core-local (no
collectives).

Transposed fp16 wire format: the rel-err budget (2e-2) dwarfs fp16
rounding (~3e-4), so the host casts to fp16 AND transposes each shard
to [256 cols, 32768 rows] before upload. On device the shard is viewed
as [128 p, 2 cb, N] — every column is a partition line, so the
per-column sum-of-squares is a native free-dim reduction and the scale
is a per-partition scalar:

  pass A: fused square+accumulate via the DVE affine_mul_reduce custom
          ucode op (out=(x*1+0)*x, accum=sum) — one DVE pass over the
          data — while HWDGE DMAs stream the next chunk.
  scale:  [128, 2] reduce + Sqrt(+eps) + reciprocal — a few hundred ns.
  pass B: y = x*s via DVE tensor_scalar_mul (cb=0) and ACT Copy with
          per-partition scale (cb=1), DMA'd out as fp16.

No TensorE, no PSUM, no broadcasts. HBM traffic = 16MB in + 16MB out
per core, the bandwidth floor; both compute passes hide under the DMA
phases. All DMAs are issued from the otherwise-idle SP queue.
"""

import numpy as np

import concourse.bacc as bacc
import concourse.bass as bass
import concourse.tile as tile
from concourse import mybir
from concourse.bass_utils import run_bass_kernel_spmd

N, D = 32768, 2048
EPS = 1e-6
NCORES = 8
C = D // NCORES  # 256 columns per core
P = 128          # partitions
CB = C // P      # 2 column blocks per partition
NK = 2048        # main chunk length along the row axis

# Pass-A chunks: big chunks first, ramp the tail down so the final
# square->scale chain is short. Pass-B chunks: ramp up so the first
# out-DMA launches right after the scale lands.
IN_CHUNKS = [NK] * 15 + [1024, 512, 256, 256]
OUT_CHUNKS = [256, 256, 512, 1024] + [NK] * 15

_NC = None


def _build() -> bass.Bass:
    nc = bacc.Bacc("TRN2", target_bir_lowering=False, enable_partition_id=False)
    x = nc.dram_tensor("x", [C, N], mybir.dt.float16, kind="ExternalInput")
    y = nc.dram_tensor("y", [C, N], mybir.dt.float16, kind="ExternalOutput")
    xv = x[:, :].rearrange("(cb p) n -> p cb n", p=P)
    yv = y[:, :].rearrange("(cb p) n -> p cb n", p=P)

    with tile.TileContext(nc) as tc:
        with (
            tc.tile_pool(name="cache", bufs=1) as cachep,
            tc.tile_pool(name="consts", bufs=1) as consts,
            tc.tile_pool(name="scr", bufs=2) as scrp,
            tc.tile_pool(name="outs", bufs=4) as outp,
            tc.tile_pool(name="scale", bufs=1) as scalep,
        ):
            xc = cachep.tile([P, CB, N], mybir.dt.float16)
            eps_t = consts.tile([P, 1], mybir.dt.float32)
            nc.vector.memset(eps_t, EPS)
            nmm = len(IN_CHUNKS)
            acc = scalep.tile([P, 2 * nmm], mybir.dt.float32)

            # Pass A: stream chunks in, fused square+reduce on two engines.
            n0 = 0
            for j, nk in enumerate(IN_CHUNKS):
                ns = slice(n0, n0 + nk)
                nc.sync.dma_start(out=xc[:, :, ns], in_=xv[:, :, ns])
                for cb in range(CB):
                    sq = scrp.tile([P, nk], mybir.dt.float16, tag=f"sq{cb}", bufs=2)
                    nc.vector.affine_mul_reduce(
                        out=sq[:, :],
                        accum_out=acc[:, 2 * j + cb : 2 * j + cb + 1],
                        in0=xc[:, cb, ns],
                        in1=xc[:, cb, ns],
                        scale=1.0,
                        bias=0.0,
                    )
                n0 += nk

            # Scale: u[p, cb] = sum of per-chunk partials; s = 1/sqrt(u+eps).
            u2 = scalep.tile([P, CB], mybir.dt.float32)
            av = acc[:, :].rearrange("p (j cb) -> p cb j", cb=2)
            nc.vector.reduce_sum(u2, av, axis=mybir.AxisListType.X)
            tsq = scalep.tile([P, CB], mybir.dt.float32)
            nc.scalar.activation(
                out=tsq[:, :],
                in_=u2[:, :],
                func=mybir.ActivationFunctionType.Sqrt,
                bias=eps_t[:, 0:1],
                scale=1.0,
            )
            s2 = scalep.tile([P, CB], mybir.dt.float32)
            nc.vector.reciprocal_approx_fast(out=s2[:, :], in_=tsq[:, :])

            # Pass B: scale cached x per column block, write fp16 out.
            n0 = 0
            for nk in OUT_CHUNKS:
                ns = slice(n0, n0 + nk)
                ot = outp.tile([P, CB, nk], mybir.dt.float16, tag="ot")
                nc.vector.tensor_scalar_mul(ot[:, 0, :], xc[:, 0, ns], s2[:, 0:1])
                nc.scalar.activation(
                    out=ot[:, 1, :],
                    in_=xc[:, 1, ns],
                    func=mybir.ActivationFunctionType.Copy,
                    scale=s2[:, 1:2],
                )
                nc.sync.dma_start(out=yv[:, :, ns], in_=ot)
                n0 += nk
    nc.compile()
    return nc


def _get_nc() -> bass.Bass:
    global _NC
    if _NC is None:
        _NC = _build()
    return _NC


def _shard_inputs(x: np.ndarray) -> list[dict]:
    xh = x.astype(np.float16)
    return [
        {"x": np.ascontiguousarray(xh[:, i * C : (i + 1) * C].T)}
        for i in range(NCORES)
    ]


def kernel(x) -> np.ndarray:
    x = np.asarray(x, dtype=np.float32)
    assert x.shape == (N, D), x.shape
    nc = _get_nc()
    in_maps = _shard_inputs(x)
    try:
        res = run_bass_kernel_spmd(nc, in_maps, core_ids=list(range(NCORES)))
    except Exception:
        # Transient NRT/device hiccups (e.g. a previous process's profiling
        # session left a core wedged) recover after a short pause.
        import time

        time.sleep(5)
        res = run_bass_kernel_spmd(nc, in_maps, core_ids=list(range(NCORES)))
    return np.concatenate(
        [r["y"].T.astype(np.float32) for r in res.results], axis=1
    )
